# revision 1
# baseline (speedup 1.0000x reference)
"""DenseCRF mean-field kernel for Trainium2 (8 NeuronCores, data parallel).

Math per sample (B=8 samples -> 1 per core):
    Q0 = softmax(unary, axis=class)
    repeat 5x:  Q <- softmax(Q - compat @ ((pos_w+bi_w) * gauss7(Q)), axis=class)
(`image` is unused by the reference math.)

The 7x7 gaussian blur is separable with reflect padding, expressed as two
banded matrix multiplies on the TensorEngine:
    pass1: T1[w, h'] = sum_h Q[h, w] * AT[h, h']              (H-conv, transposed out)
    pass2: L[h',w'] = Q[h',w'] + sum_w T1[w, h'] * (-s*AT)[w, w']  (W-conv + identity)
where AT = A.T, A the [n,n] reflect conv matrix of g, s = pos_w + bi_w
(times compat diagonal). Both passes keep the data operand stationary so the
result returns to natural [h, w] orientation; the identity matmul goes first
with start=True (clears the PSUM bank) and the overlapping band windows
accumulate per-element via PSUM has_written semantics.

Per core the sample stays resident in SBUF as fp16 between iterations; HBM
traffic is only the initial unary load and final Q store (2 x 21 MB).
"""

from contextlib import ExitStack

import numpy as np

import concourse.bacc as bacc
import concourse.tile as tile
from concourse import mybir
from concourse.bass_utils import run_bass_kernel_spmd

F32 = mybir.dt.float32
F16 = mybir.dt.float16

B, C, H, W = 8, 21, 512, 512
KSIZE, SIGMA = 7, 2.0
NUM_ITERATIONS = 5
PB = 128                       # partition block
BANDW = PB + 2 * (KSIZE // 2)  # max band window width (134)


def _gauss1d():
    coords = np.arange(KSIZE, dtype=np.float64) - KSIZE // 2
    g = np.exp(-(coords ** 2) / (2.0 * SIGMA ** 2))
    return g / g.sum()


def _conv_matrix(n, g):
    r = len(g) // 2
    A = np.zeros((n, n), np.float64)
    for i in range(n):
        for t in range(len(g)):
            j = i + t - r
            if j < 0:
                j = -j
            if j >= n:
                j = 2 * n - 2 - j
            A[i, j] += g[t]
    return A  # filt = A @ x  (reflect boundary)


def _windows(n):
    r = KSIZE // 2
    return [(max(0, PB * i - r), min(n, PB * i + PB + r)) for i in range(n // PB)]


def build_program(c=C, hb=H // PB, w=W, iters=NUM_ITERATIONS, n_cores=8,
                  b2_per_class=False, offdiag=None):
    """Build the per-core Bass program.

    offdiag: None for (scaled-)identity compat, else the full [c,c] compat
    matrix -> generic (slow) class-mix path with DRAM-resident E.
    """
    h = hb * PB
    wb = w // PB
    wins_h = _windows(h)
    wins_w = _windows(w)
    n_b2 = c if b2_per_class else 1
    generic = offdiag is not None

    nc = bacc.Bacc("TRN2", target_bir_lowering=False, debug=False,
                   num_devices=n_cores)
    U = nc.dram_tensor("unary", [c, h, w], F32, kind="ExternalInput")
    BD1 = nc.dram_tensor("band1", [hb, PB, h], F16, kind="ExternalInput")
    BD2 = nc.dram_tensor("band2", [n_b2, wb, PB, BANDW], F16, kind="ExternalInput")
    IDN = nc.dram_tensor("ident", [PB, PB], F16, kind="ExternalInput")
    OUT = nc.dram_tensor("out", [c, h, w], F32, kind="ExternalOutput")
    EDR = nc.dram_tensor("escr", [c, h, w], F16) if generic else None

    n_grp = 3 if c >= 6 else 1
    grps = np.array_split(np.arange(c), n_grp)
    grp_of, first_in_grp = {}, {}
    for gi, g in enumerate(grps):
        for k, cc in enumerate(g):
            grp_of[int(cc)] = gi
            first_in_grp[int(cc)] = (k == 0)

    with tile.TileContext(nc) as tc, ExitStack() as ctx:
        singles = ctx.enter_context(tc.tile_pool(name="singles", bufs=1))
        t1ps_pool = ctx.enter_context(tc.tile_pool(name="t1ps", bufs=2, space="PSUM"))
        lps_pool = ctx.enter_context(tc.tile_pool(name="lps", bufs=2, space="PSUM"))
        t1sb_pool = ctx.enter_context(tc.tile_pool(name="t1sb", bufs=2))
        stage_pool = ctx.enter_context(tc.tile_pool(name="stage", bufs=4))
        sums_pool = ctx.enter_context(tc.tile_pool(name="sums", bufs=2))
        mix_pool = ctx.enter_context(tc.tile_pool(name="mix", bufs=2))

        # ---- persistent / constant SBUF ----
        qres = singles.tile([PB, c, hb, w], F16, tag="qres")
        b1 = singles.tile([PB, hb, h], F16, tag="b1")
        b2 = singles.tile([PB, n_b2, wb, BANDW], F16, tag="b2")
        ident = singles.tile([PB, PB], F16, tag="ident")
        for i in range(hb):
            nc.sync.dma_start(out=b1[:, i, :], in_=BD1[i])
        for j in range(n_b2):
            for i in range(wb):
                nc.sync.dma_start(out=b2[:, j, i, :], in_=BD2[j, i])
        nc.sync.dma_start(out=ident[:], in_=IDN[:])

        spart = {}

        def accum_E_class(cc, e_ap):
            """Accumulate a full-class-width [PB, hb*w] E into group partial."""
            gi = grp_of[cc]
            if first_in_grp[cc]:
                t = sums_pool.tile([PB, hb, w], F16, tag=f"sp_{gi}")
                spart[gi] = t
                nc.vector.tensor_copy(out=t[:], in_=e_ap)
            else:
                nc.vector.tensor_add(out=spart[gi][:], in0=spart[gi][:],
                                     in1=e_ap)

        def accum_E(cc, m2, e_ap):
            gi = grp_of[cc]
            if first_in_grp[cc] and (gi, m2) not in spart:
                t = sums_pool.tile([PB, w], F16, tag=f"spm_{gi}_{m2}")
                spart[(gi, m2)] = t
                nc.vector.tensor_copy(out=t[:], in_=e_ap)
            else:
                t = spart[(gi, m2)]
                nc.vector.tensor_add(out=t[:], in0=t[:], in1=e_ap)

        def emit_exp_generic(cc, m2, src_ap):
            est = stage_pool.tile([PB, w], F16, tag="est")
            nc.scalar.activation(out=est[:], in_=src_ap,
                                 func=mybir.ActivationFunctionType.Exp)
            accum_E(cc, m2, est[:])
            nc.sync.dma_start(out=EDR[cc, m2 * PB:(m2 + 1) * PB, :],
                              in_=est[:])

        def finish_round(last):
            if not generic:
                s = sums_pool.tile([PB, hb, w], F32, tag="s", bufs=1)
                if n_grp == 1:
                    nc.vector.tensor_copy(out=s[:], in_=spart[0][:])
                else:
                    nc.vector.tensor_add(out=s[:], in0=spart[0][:],
                                         in1=spart[1][:])
                    for gi in range(2, n_grp):
                        nc.vector.tensor_add(out=s[:], in0=s[:],
                                             in1=spart[gi][:])
                r = sums_pool.tile([PB, hb, w], F32, tag="r", bufs=1)
                nc.vector.reciprocal_approx_fast(out=r[:], in_=s[:])
                rh = sums_pool.tile([PB, hb, w], F16, tag="rh")
                nc.vector.tensor_copy(out=rh[:], in_=r[:])
                for cc in range(c):
                    if not last:
                        nc.vector.tensor_mul(out=qres[:, cc], in0=qres[:, cc],
                                             in1=rh[:])
                    else:
                        fo = stage_pool.tile([PB, hb, w], F32, tag="fout", bufs=2)
                        nc.vector.tensor_mul(out=fo[:], in0=qres[:, cc],
                                             in1=rh[:])
                        # dest rows are (m2*PB + p); match fo's (p, m2, x) order
                        nc.sync.dma_start(
                            out=OUT[cc].rearrange("(m p) w -> p m w", p=PB),
                            in_=fo[:])
            else:
                rh = []
                for m2 in range(hb):
                    s = sums_pool.tile([PB, w], F32, tag=f"sm_{m2}")
                    if n_grp == 1:
                        nc.vector.tensor_copy(out=s[:], in_=spart[(0, m2)][:])
                    else:
                        nc.vector.tensor_add(out=s[:], in0=spart[(0, m2)][:],
                                             in1=spart[(1, m2)][:])
                        for gi in range(2, n_grp):
                            nc.vector.tensor_add(out=s[:], in0=s[:],
                                                 in1=spart[(gi, m2)][:])
                    r = sums_pool.tile([PB, w], F32, tag=f"rm_{m2}")
                    nc.vector.reciprocal_approx_fast(out=r[:], in_=s[:])
                    rhm = sums_pool.tile([PB, w], F16, tag=f"rhm_{m2}")
                    nc.vector.tensor_copy(out=rhm[:], in_=r[:])
                    rh.append(rhm)
                for cc in range(c):
                    for m2 in range(hb):
                        esrc = stage_pool.tile([PB, w], F16, tag="eld")
                        nc.sync.dma_start(
                            out=esrc[:], in_=EDR[cc, m2 * PB:(m2 + 1) * PB, :])
                        if not last:
                            nc.vector.tensor_mul(out=qres[:, cc, m2, :],
                                                 in0=esrc[:], in1=rh[m2][:])
                        else:
                            fo = stage_pool.tile([PB, w], F32, tag="fom")
                            nc.vector.tensor_mul(out=fo[:], in0=esrc[:],
                                                 in1=rh[m2][:])
                            nc.sync.dma_start(
                                out=OUT[cc, m2 * PB:(m2 + 1) * PB, :], in_=fo[:])
            spart.clear()

        # ---- init: Q0 = softmax(unary) ----
        for cc in range(c):
            for m2 in range(hb):
                st = stage_pool.tile([PB, w], F32, tag="uin")
                nc.sync.dma_start(out=st[:], in_=U[cc, m2 * PB:(m2 + 1) * PB, :])
                if generic:
                    emit_exp_generic(cc, m2, st[:])
                else:
                    nc.scalar.activation(out=qres[:, cc, m2, :], in_=st[:],
                                         func=mybir.ActivationFunctionType.Exp)
            if not generic:
                accum_E_class(cc, qres[:, cc])
        finish_round(last=False)

        PAIR = 2 if (hb % 2 == 0 and wb % 2 == 0 and not generic) else 1

        def emit_pass1(cc, src_fn):
            t1sb = t1sb_pool.tile([PB, wb, h], F16, tag="t1sb")
            for mp in range(0, wb, PAIR):
                t1ps = t1ps_pool.tile([PB, PAIR, h], F32, tag="t1ps")
                for ml in range(PAIR):
                    m = mp + ml
                    # first mm full-width: initializes its PSUM bank
                    # (rhs is zero outside the band), rest band windows
                    nc.tensor.matmul(
                        t1ps[:, ml, 0:h],
                        src_fn(0, slice(m * PB, (m + 1) * PB)),
                        b1[:, 0, :],
                        start=True, stop=(hb == 1))
                    for i in range(1, hb):
                        lo, hi = wins_h[i]
                        nc.tensor.matmul(
                            t1ps[:, ml, lo:hi],
                            src_fn(i, slice(m * PB, (m + 1) * PB)),
                            b1[:, i, lo:hi],
                            start=False, stop=(i == hb - 1))
                nc.scalar.copy(out=t1sb[:, mp:mp + PAIR, :], in_=t1ps[:])
            return t1sb

        def emit_pass2(cc, t1sb, last):
            b2c = b2[:, cc if n_b2 > 1 else 0]
            for m2p in range(0, hb, PAIR):
                lps = lps_pool.tile([PB, PAIR, w], F32, tag="lps")
                for ml in range(PAIR):
                    m2 = m2p + ml
                    nc.tensor.matmul(lps[:, ml, 0:w], ident[:],
                                     qres[:, cc, m2, :],
                                     start=True, stop=False)
                    for i2 in range(wb):
                        lo, hi = wins_w[i2]
                        nc.tensor.matmul(
                            lps[:, ml, lo:hi],
                            t1sb[:, i2, m2 * PB:(m2 + 1) * PB],
                            b2c[:, i2, 0:hi - lo],
                            start=False, stop=(i2 == wb - 1))
                if not generic:
                    nc.scalar.activation(
                        out=qres[:, cc, m2p:m2p + PAIR, :], in_=lps[:],
                        func=mybir.ActivationFunctionType.Exp)
                else:
                    for ml in range(PAIR):
                        emit_exp_generic(cc, m2p + ml, lps[:, ml, :])
            if not generic:
                accum_E_class(cc, qres[:, cc])

        # ---- iterations (class loop software-pipelined one deep) ----
        for k in range(iters):
            last = (k == iters - 1)
            prev = None
            for cc in range(c):
                if generic:
                    msrc = mix_pool.tile([PB, hb, w], F16, tag="mix")
                    nz = [j for j in range(c) if offdiag[cc, j] != 0.0]
                    for i in range(hb):
                        if not nz:
                            nc.vector.memset(msrc[:, i, :], 0.0)
                        else:
                            j0 = nz[0]
                            nc.vector.tensor_scalar_mul(
                                out=msrc[:, i, :], in0=qres[:, j0, i, :],
                                scalar1=float(offdiag[cc, j0]))
                            for j in nz[1:]:
                                nc.vector.scalar_tensor_tensor(
                                    out=msrc[:, i, :], in0=qres[:, j, i, :],
                                    scalar=float(offdiag[cc, j]),
                                    in1=msrc[:, i, :],
                                    op0=mybir.AluOpType.mult,
                                    op1=mybir.AluOpType.add)

                    def src_fn(i, mcols, _m=msrc):
                        return _m[:, i, mcols]
                else:
                    def src_fn(i, mcols, _c=cc):
                        return qres[:, _c, i, mcols]

                t1sb = emit_pass1(cc, src_fn)
                if prev is not None:
                    emit_pass2(prev[0], prev[1], last)
                prev = (cc, t1sb)
            emit_pass2(prev[0], prev[1], last)
            finish_round(last=last)

    nc.compile()
    return nc


def _prep_consts(c, h, w, scale, compat):
    g = _gauss1d()
    AT_h = _conv_matrix(h, g).T
    AT_w = _conv_matrix(w, g).T
    band1 = np.zeros((h // PB, PB, h), np.float16)
    for i in range(h // PB):
        band1[i] = AT_h[i * PB:(i + 1) * PB, :].astype(np.float16)

    diag = np.diag(compat).astype(np.float64)
    is_diag = bool(np.count_nonzero(compat - np.diag(diag)) == 0)
    uniform = is_diag and bool(np.all(diag == diag[0]))

    offdiag = None
    if is_diag:
        n_b2 = 1 if uniform else c
        scales = [float(scale) * float(diag[0])] if uniform else \
                 [float(scale) * float(d) for d in diag]
    else:
        n_b2 = 1
        scales = [float(scale)]
        offdiag = compat.astype(np.float64)

    band2 = np.zeros((n_b2, w // PB, PB, BANDW), np.float16)
    for j in range(n_b2):
        for i, (lo, hi) in enumerate(_windows(w)):
            band2[j, i, :, 0:hi - lo] = (
                -scales[j] * AT_w[i * PB:(i + 1) * PB, lo:hi]).astype(np.float16)
    ident = np.eye(PB, dtype=np.float16)
    return band1, band2, ident, (n_b2 > 1), offdiag


_prog_cache = {}


def kernel(unary, image, pos_w, bi_w, compatibility):
    unary = np.asarray(unary, dtype=np.float32)
    compat = np.asarray(compatibility, dtype=np.float32)
    scale = float(np.asarray(pos_w)) + float(np.asarray(bi_w))
    b, c, h, w = unary.shape
    assert (b, c, h, w) == (B, C, H, W), (b, c, h, w)

    band1, band2, ident, per_class, offdiag = _prep_consts(c, h, w, scale, compat)
    key = (scale, compat.tobytes())
    if key not in _prog_cache:
        _prog_cache[key] = build_program(
            c=c, hb=h // PB, w=w, iters=NUM_ITERATIONS, n_cores=B,
            b2_per_class=per_class, offdiag=offdiag)
    nc = _prog_cache[key]

    in_maps = [{"unary": unary[i], "band1": band1, "band2": band2,
                "ident": ident} for i in range(B)]
    res = run_bass_kernel_spmd(nc, in_maps, list(range(B)))
    out = np.stack([res.results[i]["out"] for i in range(B)], axis=0)
    return out.astype(np.float32)


if __name__ == "__main__":
    rng = np.random.default_rng(0)
    u = rng.standard_normal((B, C, H, W), dtype=np.float32)
    img = rng.random((B, 3, H, W), dtype=np.float32)
    o = kernel(u, img, np.float32(3.0), np.float32(10.0),
               np.eye(C, dtype=np.float32))
    print(o.shape, o.dtype, float(o.sum()))



# revision 4
# speedup vs baseline: 1.1187x; 1.1187x over previous
"""DenseCRF mean-field kernel for Trainium2 (8 NeuronCores, data parallel).

Math per sample (B=8 samples -> 1 per core):
    Q0 = softmax(unary, axis=class)
    repeat 5x:  Q <- softmax(Q - compat @ ((pos_w+bi_w) * gauss7(Q)), axis=class)
(`image` is unused by the reference math.)

The 7x7 gaussian blur is separable with reflect padding, expressed as two
banded matrix multiplies on the TensorEngine:
    pass1: T1[w, h'] = sum_h Q[h, w] * AT[h, h']              (H-conv, transposed out)
    pass2: L[h',w'] = Q[h',w'] + sum_w T1[w, h'] * (-s*AT)[w, w']  (W-conv + identity)
where AT = A.T, A the [n,n] reflect conv matrix of g, s = pos_w + bi_w
(times compat diagonal).

Engine balance: pass1 uses disjoint start=True column ownership (no
full-width PSUM-init matmul); the pass1 PSUM->SBUF copies are split
between the Activation and Vector engines; the softmax normalize muls and
one accumulation group run on the otherwise-idle GpSimd(Pool) engine
(SBUF-only ops). Per core the sample stays resident in SBUF as fp16
between iterations; HBM traffic is the initial unary load (f32) and the
final Q store (f16, upcast on host).
"""

from contextlib import ExitStack

import numpy as np

import concourse.bacc as bacc
import concourse.tile as tile
from concourse import mybir
from concourse.bass_utils import run_bass_kernel_spmd

F32 = mybir.dt.float32
F16 = mybir.dt.float16

B, C, H, W = 8, 21, 512, 512
KSIZE, SIGMA = 7, 2.0
NUM_ITERATIONS = 5
PB = 128                       # partition block
R = KSIZE // 2                 # band half-width (3)
BANDW = PB + 2 * R             # max band window width (134)

# engine schedules (identity-compat fast path)
POOL_ADD_GROUP = 4             # classes 0..3 accumulate on Pool
DVE_GROUPS = ((4, 10), (10, 15), (15, 20))  # three DVE class groups
# class c-1 (20) is folded directly into the final f32 sum (shortens the
# inter-iteration critical chain by one add)
POOL_MUL_FIRST = 12            # classes >= this get Pool for normalize mul
NHALF = 2                      # finish_round tail ops split into halves


def _gauss1d():
    coords = np.arange(KSIZE, dtype=np.float64) - KSIZE // 2
    g = np.exp(-(coords ** 2) / (2.0 * SIGMA ** 2))
    return g / g.sum()


def _conv_matrix(n, g):
    r = len(g) // 2
    A = np.zeros((n, n), np.float64)
    for i in range(n):
        for t in range(len(g)):
            j = i + t - r
            if j < 0:
                j = -j
            if j >= n:
                j = 2 * n - 2 - j
            A[i, j] += g[t]
    return A  # filt = A @ x  (reflect boundary)


def _windows(n):
    return [(max(0, PB * i - R), min(n, PB * i + PB + R)) for i in range(n // PB)]


def build_program(c=C, hb=H // PB, w=W, iters=NUM_ITERATIONS, n_cores=8,
                  b2_per_class=False, offdiag=None):
    """Build the per-core Bass program.

    offdiag: None for (scaled-)identity compat, else the full [c,c] compat
    matrix -> generic (slow) class-mix path with DRAM-resident E.
    """
    h = hb * PB
    wb = w // PB
    wins_h = _windows(h)
    wins_w = _windows(w)
    n_b2 = c if b2_per_class else 1
    generic = offdiag is not None

    nc = bacc.Bacc("TRN2", target_bir_lowering=False, debug=False,
                   num_devices=n_cores)
    U = nc.dram_tensor("unary", [c, h, w], F32, kind="ExternalInput")
    BD1 = nc.dram_tensor("band1", [hb, PB, h], F16, kind="ExternalInput")
    BD2 = nc.dram_tensor("band2", [n_b2, wb, PB, BANDW], F16, kind="ExternalInput")
    IDN = nc.dram_tensor("ident", [PB, PB], F16, kind="ExternalInput")
    OUT = nc.dram_tensor("out", [c, h, w], F16 if not generic else F32,
                         kind="ExternalOutput")
    EDR = nc.dram_tensor("escr", [c, h, w], F16) if generic else None

    if generic:
        n_grp = 3 if c >= 6 else 1
        grps = np.array_split(np.arange(c), n_grp)
        grp_of, first_in_grp = {}, {}
        for gi, g in enumerate(grps):
            for k, ccls in enumerate(g):
                grp_of[int(ccls)] = gi
                first_in_grp[int(ccls)] = (k == 0)
    else:
        # group 0 (classes 0..POOL_ADD_GROUP-1) accumulates on Pool; groups
        # 1..3 on DVE. Second member of each group does the pair-init add.
        grp_of, idx_in_grp = {}, {}
        bounds = [(0, POOL_ADD_GROUP)] + list(DVE_GROUPS)
        for gi, (lo, hi) in enumerate(bounds):
            for k, ccls in enumerate(range(lo, hi)):
                grp_of[ccls] = gi
                idx_in_grp[ccls] = k
        n_grp = len(bounds)

    with tile.TileContext(nc) as tc, ExitStack() as ctx:
        singles = ctx.enter_context(tc.tile_pool(name="singles", bufs=1))
        t1ps_pool = ctx.enter_context(tc.tile_pool(name="t1ps", bufs=2, space="PSUM"))
        lps_pool = ctx.enter_context(tc.tile_pool(name="lps", bufs=2, space="PSUM"))
        t1sb_pool = ctx.enter_context(tc.tile_pool(name="t1sb", bufs=2))
        stage_pool = ctx.enter_context(tc.tile_pool(name="stage", bufs=2))
        spart_pool = ctx.enter_context(tc.tile_pool(name="spart", bufs=1))
        sums_pool = ctx.enter_context(tc.tile_pool(name="sums", bufs=2))
        mix_pool = ctx.enter_context(tc.tile_pool(name="mix", bufs=2))
        out_pool = ctx.enter_context(tc.tile_pool(name="fout", bufs=3))

        # ---- persistent / constant SBUF ----
        qres = singles.tile([PB, c, hb, w], F16, tag="qres")
        b1 = singles.tile([PB, hb, h], F16, tag="b1")
        b2 = singles.tile([PB, n_b2, wb, BANDW], F16, tag="b2")
        ident = singles.tile([PB, PB], F16, tag="ident")
        for i in range(hb):
            nc.sync.dma_start(out=b1[:, i, :], in_=BD1[i])
        for j in range(n_b2):
            for i in range(wb):
                nc.sync.dma_start(out=b2[:, j, i, :], in_=BD2[j, i])
        nc.sync.dma_start(out=ident[:], in_=IDN[:])

        spart = {}
        copy_rr = [0]  # round-robin state for pass1 copy engine

        # ------------------- identity-path accumulation -------------------
        def accum_E_class(cc):
            """Accumulate E (== qres[:, cc]) into its group partial."""
            gi = grp_of[cc]
            eng = nc.gpsimd if gi == 0 else nc.vector
            k = idx_in_grp[cc]
            if k == 0:
                return  # wait for pair-init with class lo+1
            if k == 1:
                t = spart_pool.tile([PB, hb, w], F16, tag=f"sp_{gi}")
                spart[gi] = t
                eng.tensor_add(out=t[:], in0=qres[:, cc - 1], in1=qres[:, cc])
            else:
                eng.tensor_add(out=spart[gi][:], in0=spart[gi][:],
                               in1=qres[:, cc])

        # ------------------- generic-path accumulation --------------------
        def accum_E(cc, m2, e_ap):
            gi = grp_of[cc]
            if first_in_grp[cc] and (gi, m2) not in spart:
                t = sums_pool.tile([PB, w], F16, tag=f"spm_{gi}_{m2}")
                spart[(gi, m2)] = t
                nc.vector.tensor_copy(out=t[:], in_=e_ap)
            else:
                t = spart[(gi, m2)]
                nc.vector.tensor_add(out=t[:], in0=t[:], in1=e_ap)

        def emit_exp_generic(cc, m2, src_ap):
            est = stage_pool.tile([PB, w], F16, tag="est")
            nc.scalar.activation(out=est[:], in_=src_ap,
                                 func=mybir.ActivationFunctionType.Exp)
            accum_E(cc, m2, est[:])
            nc.sync.dma_start(out=EDR[cc, m2 * PB:(m2 + 1) * PB, :],
                              in_=est[:])

        def finish_round(last):
            if not generic:
                # combine tree: t0 = g0+g1, t1 = g2+g3 (f16), s = t0+t1 (f32)
                t0 = sums_pool.tile([PB, hb, w], F16, tag="t0")
                t1 = sums_pool.tile([PB, hb, w], F16, tag="t1")
                nc.vector.tensor_add(out=t0[:], in0=spart[0][:], in1=spart[1][:])
                nc.vector.tensor_add(out=t1[:], in0=spart[2][:], in1=spart[3][:])
                s = sums_pool.tile([PB, hb, w], F32, tag="s", bufs=1)
                nc.vector.tensor_add(out=s[:], in0=t0[:], in1=t1[:])
                r = sums_pool.tile([PB, hb, w], F32, tag="r", bufs=1)
                nc.vector.reciprocal_approx_fast(out=r[:], in_=s[:])
                rh = sums_pool.tile([PB, hb, w], F16, tag="rh")
                nc.vector.tensor_copy(out=rh[:], in_=r[:])
                for cc in range(c):
                    eng = nc.gpsimd if cc >= POOL_MUL_FIRST else nc.vector
                    if not last:
                        eng.tensor_mul(out=qres[:, cc], in0=qres[:, cc],
                                       in1=rh[:])
                    else:
                        fo = out_pool.tile([PB, hb, w], F16, tag="fout")
                        eng.tensor_mul(out=fo[:], in0=qres[:, cc], in1=rh[:])
                        # dest rows are (m2*PB + p); match fo's (p, m2, x) order
                        nc.sync.dma_start(
                            out=OUT[cc].rearrange("(m p) w -> p m w", p=PB),
                            in_=fo[:])
            else:
                rh = []
                for m2 in range(hb):
                    s = sums_pool.tile([PB, w], F32, tag=f"sm_{m2}")
                    if n_grp == 1:
                        nc.vector.tensor_copy(out=s[:], in_=spart[(0, m2)][:])
                    else:
                        nc.vector.tensor_add(out=s[:], in0=spart[(0, m2)][:],
                                             in1=spart[(1, m2)][:])
                        for gi in range(2, n_grp):
                            nc.vector.tensor_add(out=s[:], in0=s[:],
                                                 in1=spart[(gi, m2)][:])
                    r = sums_pool.tile([PB, w], F32, tag=f"rm_{m2}")
                    nc.vector.reciprocal_approx_fast(out=r[:], in_=s[:])
                    rhm = sums_pool.tile([PB, w], F16, tag=f"rhm_{m2}")
                    nc.vector.tensor_copy(out=rhm[:], in_=r[:])
                    rh.append(rhm)
                for cc in range(c):
                    for m2 in range(hb):
                        esrc = stage_pool.tile([PB, w], F16, tag="eld")
                        nc.sync.dma_start(
                            out=esrc[:], in_=EDR[cc, m2 * PB:(m2 + 1) * PB, :])
                        if not last:
                            nc.vector.tensor_mul(out=qres[:, cc, m2, :],
                                                 in0=esrc[:], in1=rh[m2][:])
                        else:
                            fo = stage_pool.tile([PB, w], F32, tag="fom")
                            nc.vector.tensor_mul(out=fo[:], in0=esrc[:],
                                                 in1=rh[m2][:])
                            nc.sync.dma_start(
                                out=OUT[cc, m2 * PB:(m2 + 1) * PB, :], in_=fo[:])
            spart.clear()

        # ---- init: Q0 = softmax(unary) ----
        if not generic:
            for cc in range(c):
                st = stage_pool.tile([PB, hb, w], F32, tag="uin")
                nc.sync.dma_start(
                    out=st[:], in_=U[cc].rearrange("(m p) w -> p m w", p=PB))
                nc.scalar.activation(
                    out=qres[:, cc], in_=st[:],
                    func=mybir.ActivationFunctionType.Exp)
                accum_E_class(cc)
        else:
            for cc in range(c):
                for m2 in range(hb):
                    st = stage_pool.tile([PB, w], F32, tag="uin")
                    nc.sync.dma_start(out=st[:],
                                      in_=U[cc, m2 * PB:(m2 + 1) * PB, :])
                    emit_exp_generic(cc, m2, st[:])
        finish_round(last=False)

        PAIR = 2 if (hb % 2 == 0 and wb % 2 == 0 and not generic) else 1

        def emit_pass1(cc, src_fn):
            t1sb = t1sb_pool.tile([PB, wb, h], F16, tag="t1sb")
            for mp in range(0, wb, PAIR):
                t1ps = t1ps_pool.tile([PB, PAIR, h], F32, tag="t1ps")
                for ml in range(PAIR):
                    m = mp + ml
                    # disjoint start=True ownership: block 0 owns [0, PB+R);
                    # block i>0 accumulates its R-left-overlap [PB*i-R, PB*i+R)
                    # then owns [PB*i+R, hi_i) with a fresh start=True.
                    nc.tensor.matmul(
                        t1ps[:, ml, 0:PB + R],
                        src_fn(0, slice(m * PB, (m + 1) * PB)),
                        b1[:, 0, 0:PB + R],
                        start=True, stop=(hb == 1))
                    for i in range(1, hb):
                        lo, hi = wins_h[i]
                        mid = PB * i + R
                        lhsT = src_fn(i, slice(m * PB, (m + 1) * PB))
                        nc.tensor.matmul(
                            t1ps[:, ml, lo:mid], lhsT, b1[:, i, lo:mid],
                            start=False, stop=False)
                        nc.tensor.matmul(
                            t1ps[:, ml, mid:hi], lhsT, b1[:, i, mid:hi],
                            start=True, stop=(i == hb - 1))
                # split the PSUM->SBUF copies between Act and DVE
                if generic:
                    nc.scalar.copy(out=t1sb[:, mp:mp + PAIR, :], in_=t1ps[:])
                else:
                    if copy_rr[0] % 2 == 0:
                        nc.scalar.copy(out=t1sb[:, mp:mp + PAIR, :], in_=t1ps[:])
                    else:
                        nc.vector.tensor_copy(out=t1sb[:, mp:mp + PAIR, :],
                                              in_=t1ps[:])
                    copy_rr[0] += 1
            return t1sb

        def emit_pass2(cc, t1sb, last):
            b2c = b2[:, cc if n_b2 > 1 else 0]
            for m2p in range(0, hb, PAIR):
                lps = lps_pool.tile([PB, PAIR, w], F32, tag="lps")
                for ml in range(PAIR):
                    m2 = m2p + ml
                    nc.tensor.matmul(lps[:, ml, 0:w], ident[:],
                                     qres[:, cc, m2, :],
                                     start=True, stop=False)
                    for i2 in range(wb):
                        lo, hi = wins_w[i2]
                        nc.tensor.matmul(
                            lps[:, ml, lo:hi],
                            t1sb[:, i2, m2 * PB:(m2 + 1) * PB],
                            b2c[:, i2, 0:hi - lo],
                            start=False, stop=(i2 == wb - 1))
                if not generic:
                    nc.scalar.activation(
                        out=qres[:, cc, m2p:m2p + PAIR, :], in_=lps[:],
                        func=mybir.ActivationFunctionType.Exp)
                else:
                    for ml in range(PAIR):
                        emit_exp_generic(cc, m2p + ml, lps[:, ml, :])
            if not generic:
                accum_E_class(cc)

        # ---- iterations (class loop software-pipelined one deep) ----
        for k in range(iters):
            last = (k == iters - 1)
            prev = None
            for cc in range(c):
                if generic:
                    msrc = mix_pool.tile([PB, hb, w], F16, tag="mix")
                    nz = [j for j in range(c) if offdiag[cc, j] != 0.0]
                    for i in range(hb):
                        if not nz:
                            nc.vector.memset(msrc[:, i, :], 0.0)
                        else:
                            j0 = nz[0]
                            nc.vector.tensor_scalar_mul(
                                out=msrc[:, i, :], in0=qres[:, j0, i, :],
                                scalar1=float(offdiag[cc, j0]))
                            for j in nz[1:]:
                                nc.vector.scalar_tensor_tensor(
                                    out=msrc[:, i, :], in0=qres[:, j, i, :],
                                    scalar=float(offdiag[cc, j]),
                                    in1=msrc[:, i, :],
                                    op0=mybir.AluOpType.mult,
                                    op1=mybir.AluOpType.add)

                    def src_fn(i, mcols, _m=msrc):
                        return _m[:, i, mcols]
                else:
                    def src_fn(i, mcols, _c=cc):
                        return qres[:, _c, i, mcols]

                t1sb = emit_pass1(cc, src_fn)
                if prev is not None:
                    emit_pass2(prev[0], prev[1], last)
                prev = (cc, t1sb)
            emit_pass2(prev[0], prev[1], last)
            finish_round(last=last)

    nc.compile()
    return nc


def _prep_consts(c, h, w, scale, compat):
    g = _gauss1d()
    AT_h = _conv_matrix(h, g).T
    AT_w = _conv_matrix(w, g).T
    band1 = np.zeros((h // PB, PB, h), np.float16)
    for i in range(h // PB):
        band1[i] = AT_h[i * PB:(i + 1) * PB, :].astype(np.float16)

    diag = np.diag(compat).astype(np.float64)
    is_diag = bool(np.count_nonzero(compat - np.diag(diag)) == 0)
    uniform = is_diag and bool(np.all(diag == diag[0]))

    offdiag = None
    if is_diag:
        n_b2 = 1 if uniform else c
        scales = [float(scale) * float(diag[0])] if uniform else \
                 [float(scale) * float(d) for d in diag]
    else:
        n_b2 = 1
        scales = [float(scale)]
        offdiag = compat.astype(np.float64)

    band2 = np.zeros((n_b2, w // PB, PB, BANDW), np.float16)
    for j in range(n_b2):
        for i, (lo, hi) in enumerate(_windows(w)):
            band2[j, i, :, 0:hi - lo] = (
                -scales[j] * AT_w[i * PB:(i + 1) * PB, lo:hi]).astype(np.float16)
    ident = np.eye(PB, dtype=np.float16)
    return band1, band2, ident, (n_b2 > 1), offdiag


_prog_cache = {}


def kernel(unary, image, pos_w, bi_w, compatibility):
    unary = np.asarray(unary, dtype=np.float32)
    compat = np.asarray(compatibility, dtype=np.float32)
    scale = float(np.asarray(pos_w)) + float(np.asarray(bi_w))
    b, c, h, w = unary.shape
    assert (b, c, h, w) == (B, C, H, W), (b, c, h, w)

    band1, band2, ident, per_class, offdiag = _prep_consts(c, h, w, scale, compat)
    key = (scale, compat.tobytes())
    if key not in _prog_cache:
        _prog_cache[key] = build_program(
            c=c, hb=h // PB, w=w, iters=NUM_ITERATIONS, n_cores=B,
            b2_per_class=per_class, offdiag=offdiag)
    nc = _prog_cache[key]

    in_maps = [{"unary": unary[i], "band1": band1, "band2": band2,
                "ident": ident} for i in range(B)]
    res = run_bass_kernel_spmd(nc, in_maps, list(range(B)))
    out = np.stack([res.results[i]["out"] for i in range(B)], axis=0)
    return out.astype(np.float32)


if __name__ == "__main__":
    rng = np.random.default_rng(0)
    u = rng.standard_normal((B, C, H, W), dtype=np.float32)
    img = rng.random((B, 3, H, W), dtype=np.float32)
    o = kernel(u, img, np.float32(3.0), np.float32(10.0),
               np.eye(C, dtype=np.float32))
    print(o.shape, o.dtype, float(o.sum()))
